# revision 8
# baseline (speedup 1.0000x reference)
"""Distributed Trainium2 kernel for nn_ActionEmbeddingModel.

Reference computation (B=4096, DC=1024, A=20000, C=128, H=1024):
    h         = relu(context @ w1 + b1)          # [B, H]
    ctx_score = h @ w2[:H]                       # [B]
    act_score = emb @ w2[H:]                     # [A]
    out[b, a] = ctx_score[b] + act_score[a] + b2 # [B, A]

Sharding (8 cores): data-parallel over the batch for context/h/ctx_score;
emb is sharded over actions, each core computes its act_score shard and an
AllGather replicates the full [A] action-score row. The [B/8, A] output
shard is generated PE-free: the act row is partition-broadcast on GpSimd
and per-batch-row scores are added as per-partition scalars on DVE/ACT,
so the output phase is purely DMA-bound.

Matmuls run in float32r (fp32 bits, single-pass PE streaming, ~1.5e-4
matmul rel err vs 4-cycle/row exact fp32). Host-side prep only reorders
memory (transposes / reshapes); all FLOPs run on device.
"""

import numpy as np

import concourse.bass as bass
import concourse.bass_isa as bass_isa
import concourse.mybir as mybir
from concourse import bacc
import concourse.tile as tile
from concourse.tile import TileContext
from concourse.bass_utils import run_bass_kernel_spmd

# Problem shape (hardcoded per harness contract).
B, DC, A, C, H = 4096, 1024, 20000, 128, 1024
N_CORES = 8
B_SH = B // N_CORES        # 512 batch rows per core
A_SH = A // N_CORES        # 2500 actions per core (emb shard)
P = 128                    # partitions
KT = DC // P               # 8 contraction tiles for fc1
HT = H // P                # 8 hidden tiles
BT = B_SH // P             # 4 batch chunks of 128 rows
FCH = 5000                 # output free-dim super-chunk (per DMA)
NF = A // FCH              # 4 super-chunks
MM_N = 500                 # matmul free-dim chunk (<=512 fp32)
F32 = mybir.dt.float32
F32R = mybir.dt.float32r

_CACHED_NC = None


def _build():
    nc = bacc.Bacc(num_devices=N_CORES)

    ctxT = nc.declare_dram_parameter("ctxT", [DC, B_SH], F32R, isOutput=False)
    w1 = nc.declare_dram_parameter("w1", [DC, H], F32R, isOutput=False)
    b1c = nc.declare_dram_parameter("b1c", [P, HT], F32, isOutput=False)
    w2h = nc.declare_dram_parameter("w2h", [P, HT], F32R, isOutput=False)
    w2c = nc.declare_dram_parameter("w2c", [P, 1], F32R, isOutput=False)
    b2 = nc.declare_dram_parameter("b2", [1, 1], F32, isOutput=False)
    embT = nc.declare_dram_parameter("embT", [C, A_SH], F32R, isOutput=False)
    one1 = nc.declare_dram_parameter("one1", [1, 1], F32, isOutput=False)
    out_ext = nc.declare_dram_parameter("out", [B_SH, A], F32, isOutput=True)

    # Collective bounce buffers (collectives can't touch I/O tensors).
    ag_in = nc.dram_tensor("ag_in", [A_SH], F32)
    ag_out = nc.dram_tensor("ag_out", [A], F32, addr_space="Shared")

    w1r = w1.rearrange("(kt p) h -> kt p h", p=P)
    ctxTr = ctxT.rearrange("(kt p) n -> kt p n", p=P)
    relu = mybir.ActivationFunctionType.Relu

    with TileContext(nc, num_cores=N_CORES) as tc:
        with tc.tile_pool(name="persist", bufs=1) as persist:
            ctx_col = persist.tile([P, BT], F32, tag="ctx_col")

            with (
                tc.tile_pool(name="pro", bufs=1) as pro,
                tc.tile_pool(name="pro_psum", bufs=4, space="PSUM") as pp,
                tc.tile_pool(name="pro_psum1", bufs=2, space="PSUM") as pp1,
                tc.tile_pool(name="tr_psum", bufs=1, space="PSUM") as trp,
            ):
                # ---- input DMAs ----
                emb_sb = pro.tile([C, A_SH], F32R, tag="emb")
                nc.scalar.dma_start(out=emb_sb[:, :], in_=embT[:, :])
                w2c_sb = pro.tile([P, 1], F32R, tag="w2c")
                nc.scalar.dma_start(out=w2c_sb[:, :], in_=w2c[:, :])
                b1_sb = pro.tile([P, HT], F32, tag="b1")
                nc.scalar.dma_start(out=b1_sb[:, :], in_=b1c[:, :])
                w2h_sb = pro.tile([P, HT], F32R, tag="w2h")
                nc.scalar.dma_start(out=w2h_sb[:, :], in_=w2h[:, :])
                b2_sb = pro.tile([1, 1], F32, tag="b2")
                nc.scalar.dma_start(out=b2_sb[:, :], in_=b2[:, :])
                one_sb = pro.tile([1, 1], F32, tag="one1")
                nc.scalar.dma_start(out=one_sb[:, :], in_=one1[:, :])
                w1_sb = pro.tile([P, KT * H], F32R, tag="w1")
                ctx_sb = pro.tile([P, KT * B_SH], F32R, tag="ctx")
                for kt in range(KT):
                    nc.sync.dma_start(
                        out=w1_sb[:, kt * H:(kt + 1) * H], in_=w1r[kt, :, :]
                    )
                    nc.sync.dma_start(
                        out=ctx_sb[:, kt * B_SH:(kt + 1) * B_SH],
                        in_=ctxTr[kt, :, :],
                    )

                # ---- act_score shard = embT.T @ w2c + b2 (f32r), then AllGather ----
                # GpSimd library warm-up so the first real bcast is hot.
                warm = pro.tile([P, 8], F32, tag="warm")
                nc.gpsimd.partition_broadcast(warm[:, :], b1_sb[0:1, 0:8])
                act_sb = pro.tile([1, A_SH], F32, tag="act")
                act_mms = []
                for at in range(A_SH // MM_N):
                    ps = pp1.tile([1, MM_N], F32, tag="act_ps")
                    act_mms.append(nc.tensor.matmul(
                        ps[:, :],
                        w2c_sb[:, :],
                        emb_sb[:, at * MM_N:(at + 1) * MM_N],
                        start=True,
                        stop=True,
                    ))
                    nc.scalar.add(
                        act_sb[:, at * MM_N:(at + 1) * MM_N],
                        ps[:, :],
                        b2_sb[0:1, 0:1],
                    )
                nc.gpsimd.dma_start(out=ag_in[None, :], in_=act_sb[0:1, :])
                nc.gpsimd.collective_compute(
                    "AllGather",
                    mybir.AluOpType.bypass,
                    replica_groups=[list(range(N_CORES))],
                    ins=[ag_in[:]],
                    outs=[ag_out[:]],
                )

                # ---- hT = relu(w1.T @ ctx.T + b1): HT tiles of [128, B_SH] ----
                ht_tiles = []
                first_ht_mm = None
                for ht in range(HT):
                    ps = pp.tile([P, B_SH], F32, tag="h_ps")
                    for kt in range(KT):
                        mm = nc.tensor.matmul(
                            ps[:, :],
                            w1_sb[:, kt * H + ht * P:kt * H + (ht + 1) * P],
                            ctx_sb[:, kt * B_SH:(kt + 1) * B_SH],
                            start=(kt == 0),
                            stop=(kt == KT - 1),
                        )
                        if first_ht_mm is None:
                            first_ht_mm = mm
                            tile.add_dep_helper(
                                first_ht_mm.ins,
                                act_mms[-1].ins,
                                sync=False,
                                reason="act matvecs first on PE",
                            )
                    hts = pro.tile([P, B_SH], F32R, tag=f"ht{ht}")
                    nc.scalar.activation(
                        hts[:, :], ps[:, :], relu, bias=b1_sb[:, ht:ht + 1]
                    )
                    ht_tiles.append(hts)

                # ---- ctx_score row [1, B_SH] = w2[:H].T @ hT ----
                psc = pp1.tile([1, B_SH], F32, tag="act_ps")
                for ht in range(HT):
                    nc.tensor.matmul(
                        psc[:, :],
                        w2h_sb[:, ht:ht + 1],
                        ht_tiles[ht][:, :],
                        start=(ht == 0),
                        stop=(ht == HT - 1),
                    )
                ctx_row = pro.tile([1, B_SH], F32, tag="ctx_row")
                nc.vector.tensor_copy(ctx_row[:, :], psc[:, :])

                # ---- transpose ctx_row -> ctx_col [128, BT] via K=1 matmuls ----
                for bs in range(BT):
                    pst = trp.tile([P, 1], F32, tag="tr_ps")
                    nc.tensor.matmul(
                        pst[:, :],
                        ctx_row[0:1, bs * P:(bs + 1) * P],
                        one_sb[0:1, 0:1],
                        start=True,
                        stop=True,
                    )
                    nc.scalar.copy(ctx_col[:, bs:bs + 1], pst[:, :])

            # ---- output: out[bs, f] = bcast(act)[:, f] + ctx_col[bs] ----
            with (
                tc.tile_pool(name="outp", bufs=3) as outp,
                tc.tile_pool(name="actp", bufs=1) as actp,
                tc.tile_pool(name="abcp", bufs=2) as abcp,
            ):
                act_rows = []
                for f in range(NF):
                    act_row = actp.tile([1, FCH], F32, tag=f"arow{f}")
                    nc.scalar.dma_start(
                        out=act_row[:, :],
                        in_=ag_out[None, f * FCH:(f + 1) * FCH],
                    )
                    act_rows.append(act_row)
                for f in range(NF):
                    act_bc = abcp.tile([P, FCH], F32, tag="abc")
                    nc.gpsimd.partition_broadcast(
                        act_bc[:, :], act_rows[f][0:1, :]
                    )
                    for bs in range(BT):
                        o_sb = outp.tile([P, FCH], F32, tag="osb")
                        if (f * BT + bs) % 2:
                            nc.scalar.activation(
                                o_sb[:, :],
                                act_bc[:, :],
                                mybir.ActivationFunctionType.Identity,
                                bias=ctx_col[:, bs:bs + 1],
                            )
                        else:
                            nc.vector.tensor_scalar_add(
                                o_sb[:, :], act_bc[:, :], ctx_col[:, bs:bs + 1]
                            )
                        nc.sync.dma_start(
                            out=out_ext[
                                bs * P:(bs + 1) * P, f * FCH:(f + 1) * FCH
                            ],
                            in_=o_sb[:, :],
                        )
    nc.finalize()
    return nc


def _get_nc():
    global _CACHED_NC
    if _CACHED_NC is None:
        _CACHED_NC = _build()
    return _CACHED_NC


def _in_maps(context, w1, b1, emb, w2, b2):
    context = np.asarray(context, dtype=np.float32)
    w1 = np.asarray(w1, dtype=np.float32)
    b1 = np.asarray(b1, dtype=np.float32)
    emb = np.asarray(emb, dtype=np.float32)
    w2 = np.asarray(w2, dtype=np.float32)
    b2 = np.asarray(b2, dtype=np.float32)

    b1c = np.ascontiguousarray(b1.reshape(HT, P).T)
    w2h = np.ascontiguousarray(w2[:H].reshape(HT, P).T)
    w2c = np.ascontiguousarray(w2[H:].reshape(P, 1))
    b2m = b2.reshape(1, 1)
    one1 = np.ones((1, 1), dtype=np.float32)

    maps = []
    for i in range(N_CORES):
        ctx_sh = np.ascontiguousarray(context[i * B_SH:(i + 1) * B_SH].T)
        emb_sh = np.ascontiguousarray(emb[i * A_SH:(i + 1) * A_SH].T)
        maps.append(
            {
                "ctxT": ctx_sh,
                "w1": w1,
                "b1c": b1c,
                "w2h": w2h,
                "w2c": w2c,
                "b2": b2m,
                "embT": emb_sh,
                "one1": one1,
            }
        )
    return maps


def kernel(context, w1, b1, emb, w2, b2, _trace=False, **_trace_kwargs):
    nc = _get_nc()
    maps = _in_maps(context, w1, b1, emb, w2, b2)
    res = run_bass_kernel_spmd(
        nc, maps, core_ids=list(range(N_CORES)), trace=_trace, **_trace_kwargs
    )
    out = np.concatenate([res.results[i]["out"] for i in range(N_CORES)], axis=0)
    if _trace:
        return out, res
    return out
